# revision 6
# baseline (speedup 1.0000x reference)
# Trainium2 Bass kernel for nn_ASAP (gnn_message_passing).
# Strategy: edges are partitioned by destination node across 8 cores
# (round-robin by degree rank so all cores share one SPMD schedule).
# Message MLPs run on-device in feature-major [128, F] tiles (2 groups of
# 64 features); segment-max is a log-tree of tensor_tensor(max) ops over
# per-destination windows; the source contribution enters through a
# host-prepared per-edge feature array (pure function of edge_index), the
# destination-side hidden state h1 is broadcast per segment with an
# identity-matmul over a step-0 access pattern. BatchNorm (eval, gamma=1,
# beta=0) folds into the weights. Final mean-pool + 2-layer head run on
# host from the small per-core h tables.
import math
import numpy as np
import ml_dtypes

import concourse.bass as bass
import concourse.bacc as bacc
import concourse.mybir as mybir
import concourse.tile as tile
from concourse.bass_utils import run_bass_kernel_spmd
from concourse._compat import with_exitstack

NCORES = 8
HID = 64
FMAX = 1024
INV_STD = np.float32(1.0 / np.sqrt(1.0 + 1e-5))
BF16 = ml_dtypes.bfloat16
FP32 = np.float32
TRACE = False
_LAST_RES = None


def _fold(params):
    out = {}
    for cv in ("conv1", "conv2"):
        blocks = []
        for blk in params[cv]:
            W = np.asarray(blk["W"], FP32) * INV_STD
            b = np.asarray(blk["b"], FP32) * INV_STD
            blocks.append((W, b))
        out[cv] = blocks
    return out


def _schedule(deg_by_core):
    """deg_by_core: list of per-core descending degree arrays.
    Returns (tiles, S, C): tiles = (w, nseg, ocol, ccol, i0)."""
    NN = max(len(d) for d in deg_by_core)
    prof = np.zeros(NN, np.int64)
    for d in deg_by_core:
        prof[: len(d)] = np.maximum(prof[: len(d)], d)
    tiles = []
    i, ocol, ccol = 0, 0, 0
    while i < NN:
        w = int(prof[i])
        rem = NN - i
        nseg = max(1, min(FMAX // w, (rem + 1) // 2))
        tiles.append((w, nseg, ocol, ccol, i))
        ocol += 2 * nseg * w
        ccol += nseg
        i += min(2 * nseg, rem)
    return tiles, ocol, ccol


def _emit_tree_max(nc, pool_out, zflat, z3s, tmpA, tmpB, nseg, w, use_act):
    """pool_out[128, nseg] = max over w of zflat [128, nseg*w] (PSUM).
    First evacuates PSUM to bf16 SBUF (TensorTensor cannot read 2 PSUM
    operands), then runs a log-tree of max ops on SBUF."""
    op = mybir.AluOpType.max
    F = nseg * w
    if w == 1:
        nc.vector.tensor_copy(pool_out, zflat[:, 0:F])
        return
    if use_act:
        nc.scalar.activation(z3s[:, 0:F], zflat[:, 0:F],
                             mybir.ActivationFunctionType.Copy)
    else:
        nc.vector.tensor_copy(z3s[:, 0:F], zflat[:, 0:F])
    src3 = z3s[:, 0:F].rearrange("p (n w) -> p n w", w=w)
    cur, cw, use_a = src3, w, True
    while cw > 1:
        m = 1 << (cw.bit_length() - 1)
        if m == cw:
            m //= 2
        rem = cw - m
        if m == 1:
            dst = pool_out.rearrange("p (n o) -> p n o", o=1)
        else:
            flat = tmpA if use_a else tmpB
            dst = flat[:, 0 : nseg * m].rearrange("p (n q) -> p n q", q=m)
            use_a = not use_a
        nc.vector.tensor_tensor(dst[:, :, 0:rem], cur[:, :, 0:rem], cur[:, :, m:cw], op=op)
        if m > rem:
            nc.vector.tensor_copy(dst[:, :, rem:m], cur[:, :, rem:m])
        cur, cw = dst, m


def _emit_bcast_mm(nc, z, c2h, ident, ccol, nseg, w, half, start):
    """Accumulate c2h[:, ccol+s] broadcast over window w into z[64h:64h+64, :],
    splitting matmuls at PSUM bank boundaries (512 cols). Last MM closes the
    accumulation group (stop=True)."""
    F = nseg * w
    p0 = 64 * half
    plan = []
    for c0 in range(0, F, 512):
        c1 = min(c0 + 512, F)
        s0, r0 = divmod(c0, w)
        pos = c0
        if r0:
            take = min(w - r0, c1 - c0)
            plan.append((pos, take, s0, 1, take))
            pos += take
            s0 += 1
        nfull = (c1 - pos) // w
        if nfull > 0:
            plan.append((pos, nfull * w, s0, nfull, w))
            pos += nfull * w
            s0 += nfull
        if pos < c1:
            take = c1 - pos
            plan.append((pos, take, s0, 1, take))
    for k, (pos, ncols, s0, nsegs, wk) in enumerate(plan):
        bc = c2h[p0 : p0 + 64, ccol + s0 : ccol + s0 + nsegs].to_broadcast([64, nsegs, wk])
        nc.tensor.matmul(z[p0 : p0 + 64, pos : pos + ncols], ident[p0 : p0 + 64, :], bc,
                         start=start, stop=(k == len(plan) - 1),
                         tile_position=(p0, p0), skip_group_check=True)


def _build_program(tiles, S, C):
    nc = bacc.Bacc("TRN2", target_bir_lowering=False, debug=False,
                   num_devices=NCORES)
    fp32, bf16 = mybir.dt.float32, mybir.dt.bfloat16
    Relu = mybir.ActivationFunctionType.Relu
    Copy = mybir.ActivationFunctionType.Copy

    mi_d = nc.dram_tensor("msgin", [16, S], bf16, kind="ExternalInput")
    wdefs = [("w1l1", [16, HID]), ("w1l2", [128, HID]), ("w1l3", [128, HID]),
             ("w2l1", [16, HID]), ("w2l2", [128, HID]), ("w2l3", [128, HID]),
             ("wh2", [128, HID]), ("identd", [128, HID])]
    wd = {n: nc.dram_tensor(n, s, bf16, kind="ExternalInput") for n, s in wdefs}
    bdefs = ["b11", "b21", "b31", "b12", "b22"]
    bd = {n: nc.dram_tensor(n, [128, 1], fp32, kind="ExternalInput") for n in bdefs}
    h1_o = nc.dram_tensor("h1", [128, C], bf16, kind="ExternalOutput")
    p2_o = nc.dram_tensor("p2", [128, C], fp32, kind="ExternalOutput")

    with tile.TileContext(nc) as tc:
        with tc.tile_pool(name="wp", bufs=1) as wp, \
             tc.tile_pool(name="per", bufs=1) as per, \
             tc.tile_pool(name="mip", bufs=3) as mip, \
             tc.tile_pool(name="mp", bufs=2) as mp, \
             tc.tile_pool(name="trp", bufs=1) as trp, \
             tc.tile_pool(name="zp", bufs=3, space="PSUM") as zp, \
             tc.tile_pool(name="cp", bufs=2, space="PSUM") as cp:
            wt = {}
            for n, s in wdefs:
                t = wp.tile(s, bf16, tag=n)
                nc.sync.dma_start(t[:, :], wd[n].ap())
                wt[n] = t
            bt = {}
            for n in bdefs:
                t = wp.tile([128, 1], fp32, tag=n)
                nc.sync.dma_start(t[:, :], bd[n].ap())
                bt[n] = t

            pst1 = per.tile([128, C], fp32, tag="pst1")
            h1t = per.tile([128, C], bf16, tag="h1t")
            c2h = per.tile([128, C], bf16, tag="c2h")
            pst2 = per.tile([128, C], fp32, tag="pst2")
            tmpA = trp.tile([128, FMAX], bf16, tag="tmpA")
            tmpB = trp.tile([128, FMAX], bf16, tag="tmpB")

            def conv(wl1, wl2, wl3, b1, b2, pst, use_c2h):
                for (w, nseg, ocol, ccol, _i0) in tiles:
                    F = nseg * w
                    mit = mip.tile([16, 2 * FMAX], bf16, tag="mi")
                    nc.sync.dma_start(mit[:, 0 : 2 * F], mi_d.ap()[:, ocol : ocol + 2 * F])
                    z1 = zp.tile([128, FMAX], fp32, tag="z")
                    for h in (0, 1):
                        for c0 in range(0, F, 512):
                            cw = min(512, F - c0)
                            nc.tensor.matmul(
                                z1[64 * h : 64 * h + 64, c0 : c0 + cw], wl1[:, :],
                                mit[:, h * F + c0 : h * F + c0 + cw],
                                start=True, stop=not use_c2h,
                                tile_position=(0, 64 * h), skip_group_check=True)
                    if use_c2h:
                        for h in (0, 1):
                            _emit_bcast_mm(nc, z1, c2h, wt["identd"], ccol, nseg, w, h, False)
                    m1 = mp.tile([128, FMAX], bf16, tag="m1")
                    nc.scalar.activation(m1[:, 0:F], z1[:, 0:F], Relu, bias=b1[:, 0:1])
                    z2 = zp.tile([128, FMAX], fp32, tag="z")
                    for h in (0, 1):
                        for c0 in range(0, F, 512):
                            cw = min(512, F - c0)
                            nc.tensor.matmul(
                                z2[64 * h : 64 * h + 64, c0 : c0 + cw],
                                wl2[64 * h : 64 * h + 64, :],
                                m1[64 * h : 64 * h + 64, c0 : c0 + cw],
                                start=True, stop=True,
                                tile_position=(64 * h, 64 * h), skip_group_check=True)
                    m2 = mp.tile([128, FMAX], bf16, tag="m2")
                    nc.scalar.activation(m2[:, 0:F], z2[:, 0:F], Relu, bias=b2[:, 0:1])
                    z3 = zp.tile([128, FMAX], fp32, tag="z")
                    for h in (0, 1):
                        for c0 in range(0, F, 512):
                            cw = min(512, F - c0)
                            nc.tensor.matmul(
                                z3[64 * h : 64 * h + 64, c0 : c0 + cw],
                                wl3[64 * h : 64 * h + 64, :],
                                m2[64 * h : 64 * h + 64, c0 : c0 + cw],
                                start=True, stop=True,
                                tile_position=(64 * h, 64 * h), skip_group_check=True)
                    z3s = mp.tile([128, FMAX], bf16, tag="z3s")
                    _emit_tree_max(nc, pst[:, ccol : ccol + nseg], z3[:, 0:F],
                                   z3s, tmpA, tmpB, nseg, w, use_act=(ccol % 2 == 0))

            conv(wt["w1l1"][:, :], wt["w1l2"][:, :], wt["w1l3"][:, :],
                 bt["b11"], bt["b21"], pst1, False)
            # h1 = relu(pool + b3) (bf16), also DMA out
            nc.scalar.activation(h1t[:, :], pst1[:, :], Relu, bias=bt["b31"][:, 0:1])
            nc.sync.dma_start(h1_o.ap(), h1t[:, :])
            # c2h = Wh2.T @ h1 per node column
            for c0 in range(0, C, 512):
                cw = min(512, C - c0)
                zc = cp.tile([128, 512], fp32, tag="zc")
                for h in (0, 1):
                    nc.tensor.matmul(zc[64 * h : 64 * h + 64, 0:cw],
                                     wt["wh2"][64 * h : 64 * h + 64, :],
                                     h1t[64 * h : 64 * h + 64, c0 : c0 + cw],
                                     start=True, stop=True,
                                     tile_position=(64 * h, 64 * h), skip_group_check=True)
                nc.scalar.activation(c2h[:, c0 : c0 + cw], zc[:, 0:cw], Copy)
            conv(wt["w2l1"][:, :], wt["w2l2"][:, :], wt["w2l3"][:, :],
                 bt["b12"], bt["b22"], pst2, True)
            nc.sync.dma_start(p2_o.ap(), pst2[:, :])
    nc.compile()
    return nc


def _prepare(x, pos, edge_index, batch):
    N = x.shape[0]
    src = np.asarray(edge_index[0], np.int64)
    dst = np.asarray(edge_index[1], np.int64)
    deg = np.bincount(dst, minlength=N)
    eorder = np.argsort(dst, kind="stable")
    estart = np.zeros(N + 1, np.int64)
    estart[1:] = np.cumsum(deg)
    nz = np.nonzero(deg > 0)[0]
    nzo = nz[np.argsort(-deg[nz], kind="stable")]
    core_nodes = [nzo[c::NCORES] for c in range(NCORES)]
    deg_by_core = [deg[cn] for cn in core_nodes]
    tiles, S, C = _schedule(deg_by_core)

    msgin = np.zeros((NCORES, 16, S), np.float32)
    # node_at: [core, half, C] -> node id or -1 (dup/virtual)
    node_at = np.full((NCORES, 2, C), -1, np.int64)
    for c in range(NCORES):
        cn = core_nodes[c]
        eids = np.full(S, -1, np.int64)
        for (w, nseg, ocol, ccol, i0) in tiles:
            avail = len(cn) - i0
            if avail <= 0:
                continue
            nA = min(nseg, avail)
            nB = min(nseg, max(0, avail - nseg))
            for half, nreal in ((0, nA), (1, nB)):
                if nreal == 0:
                    continue
                nodes = cn[i0 + half * nseg : i0 + half * nseg + nreal]
                node_at[c, half, ccol : ccol + nreal] = nodes
                padded = np.concatenate([nodes, np.repeat(nodes[-1:], nseg - nreal)])
                d = deg[padded]
                offs = np.minimum(np.arange(w)[None, :], (d - 1)[:, None])
                ids = eorder[estart[padded][:, None] + offs]  # [nseg, w]
                base = ocol + half * nseg * w
                eids[base : base + nseg * w] = ids.ravel()
        valid = eids >= 0
        e = eids[valid]
        dv, sv = dst[e], src[e]
        mi = msgin[c]
        mi[0:3, valid] = pos[dv].T
        mi[3:6, valid] = (pos[sv] - pos[dv]).T
        mi[6:9, valid] = x[dv].T
    return tiles, S, C, msgin.astype(BF16), node_at, deg


def kernel(x, pos, edge_index, batch, params):
    x = np.asarray(x, FP32)
    pos = np.asarray(pos, FP32)
    batch = np.asarray(batch, np.int64)
    fold = _fold(params)
    tiles, S, C, msgin, node_at, deg = _prepare(x, pos, edge_index, batch)
    nc = _build_program(tiles, S, C)

    (W11, b11), (W12, b21), (W13, b31) = fold["conv1"]
    (W21, b12), (W22, b22), (W23, _b32_unused) = fold["conv2"]
    b32 = fold["conv2"][2][1]

    def pad16(W):
        out = np.zeros((16, HID), np.float32)
        out[: W.shape[0]] = W
        return out.astype(BF16)

    def stack2(b):
        return np.concatenate([b, b]).astype(FP32).reshape(128, 1)

    def stackw(W):
        return np.concatenate([W, W], axis=0).astype(BF16)

    shared = {
        "w1l1": pad16(W11), "w1l2": stackw(W12), "w1l3": stackw(W13),
        "w2l1": pad16(W21[0:6]), "w2l2": stackw(W22), "w2l3": stackw(W23),
        "wh2": stackw(W21[6:70]),
        "identd": stackw(np.eye(HID, dtype=np.float32)),
        "b11": stack2(b11), "b21": stack2(b21), "b31": stack2(b31),
        "b12": stack2(b12), "b22": stack2(b22),
    }
    in_maps = [dict(shared, msgin=msgin[c]) for c in range(NCORES)]
    res = run_bass_kernel_spmd(nc, in_maps, core_ids=list(range(NCORES)))
    globals()["_LAST_RES"] = res
    globals()["_LAST_NC"] = nc
    globals()["_LAST_INMAPS"] = in_maps

    # host: mean pool + head
    B = int(batch.max()) + 1
    cnt = np.bincount(batch, minlength=B).astype(FP32)
    pool1 = np.zeros((B, HID), FP32)
    pool2 = np.zeros((B, HID), FP32)
    for c in range(NCORES):
        h1 = np.asarray(res.results[c]["h1"], dtype=FP32)
        p2 = np.asarray(res.results[c]["p2"], dtype=FP32)
        h2 = np.maximum(p2 + np.concatenate([b32, b32]).reshape(128, 1), 0.0)
        for half in (0, 1):
            sel = node_at[c, half] >= 0
            if not sel.any():
                continue
            g = batch[node_at[c, half, sel]]
            np.add.at(pool1, g, h1[64 * half : 64 * half + 64, sel].T)
            np.add.at(pool2, g, h2[64 * half : 64 * half + 64, sel].T)
    pool1 /= cnt[:, None]
    pool2 /= cnt[:, None]
    hcat = np.concatenate([pool1, pool2], axis=1)
    W1 = np.asarray(params["lin1"]["W"], FP32); bl1 = np.asarray(params["lin1"]["b"], FP32)
    W2 = np.asarray(params["lin2"]["W"], FP32); bl2 = np.asarray(params["lin2"]["b"], FP32)
    h = np.maximum(hcat @ W1 + bl1, 0.0)
    logits = h @ W2 + bl2
    logits = logits - logits.max(axis=1, keepdims=True)
    lse = np.log(np.exp(logits).sum(axis=1, keepdims=True))
    return (logits - lse).astype(FP32)


# revision 9
# speedup vs baseline: 1.1215x; 1.1215x over previous
# Trainium2 Bass kernel for nn_ASAP (gnn_message_passing).
# Strategy: edges are partitioned by destination node across 8 cores
# (round-robin by degree rank so all cores share one SPMD schedule).
# Message MLPs run on-device in feature-major [128, F] tiles (2 groups of
# 64 features); segment-max is a log-tree of tensor_tensor(max) ops over
# per-destination windows; the source contribution enters through a
# host-prepared per-edge feature array (pure function of edge_index), the
# destination-side hidden state h1 is broadcast per segment with an
# identity-matmul over a step-0 access pattern. BatchNorm (eval, gamma=1,
# beta=0) folds into the weights. Final mean-pool + 2-layer head run on
# host from the small per-core h tables.
import math
import numpy as np
import ml_dtypes

import concourse.bass as bass
import concourse.bacc as bacc
import concourse.mybir as mybir
import concourse.tile as tile
from concourse.bass_utils import run_bass_kernel_spmd
from concourse._compat import with_exitstack

NCORES = 8
HID = 64
FMAX = 1024
INV_STD = np.float32(1.0 / np.sqrt(1.0 + 1e-5))
BF16 = ml_dtypes.bfloat16
FP32 = np.float32
TRACE = False
_LAST_RES = None
# perf tunables (cost-model guided)
Z3_EVAC = "alt"      # "act" | "dve" | "alt"
RELU2_DVE_ALT = False  # alternate relu2 onto DVE for odd tiles
MP_BUFS = 2
ZP_BUFS = 3
MIP_BUFS = 3
SKIP_CONV2 = False
SKIP_TREE = False
SKIP_L1MM = False


def _fold(params):
    out = {}
    for cv in ("conv1", "conv2"):
        blocks = []
        for blk in params[cv]:
            W = np.asarray(blk["W"], FP32) * INV_STD
            b = np.asarray(blk["b"], FP32) * INV_STD
            blocks.append((W, b))
        out[cv] = blocks
    return out


def _schedule(deg_by_core):
    """deg_by_core: list of per-core descending degree arrays.
    Returns (tiles, S, C): tiles = (w, nseg, ocol, ccol, i0)."""
    NN = max(len(d) for d in deg_by_core)
    prof = np.zeros(NN, np.int64)
    for d in deg_by_core:
        prof[: len(d)] = np.maximum(prof[: len(d)], d)
    tiles = []
    i, ocol, ccol = 0, 0, 0
    while i < NN:
        w = int(prof[i])
        rem = NN - i
        nseg = max(1, min(FMAX // w, (rem + 1) // 2))
        tiles.append((w, nseg, ocol, ccol, i))
        ocol += nseg * w
        ccol += nseg
        i += min(2 * nseg, rem)
    return tiles, ocol, ccol


def _emit_tree_max(nc, pool_out, zflat, z3s, tmpA, tmpB, nseg, w, use_act):
    """pool_out[128, nseg] = max over w of zflat [128, nseg*w] (PSUM).
    First evacuates PSUM to bf16 SBUF (TensorTensor cannot read 2 PSUM
    operands), then runs a log-tree of max ops on SBUF."""
    op = mybir.AluOpType.max
    F = nseg * w
    if w == 1:
        nc.vector.tensor_copy(pool_out, zflat[:, 0:F])
        return
    if use_act:
        nc.scalar.activation(z3s[:, 0:F], zflat[:, 0:F],
                             mybir.ActivationFunctionType.Copy)
    else:
        nc.vector.tensor_copy(z3s[:, 0:F], zflat[:, 0:F])
    src3 = z3s[:, 0:F].rearrange("p (n w) -> p n w", w=w)
    cur, cw, use_a = src3, w, True
    while cw > 1:
        m = 1 << (cw.bit_length() - 1)
        if m == cw:
            m //= 2
        rem = cw - m
        if m == 1:
            dst = pool_out.rearrange("p (n o) -> p n o", o=1)
        else:
            flat = tmpA if use_a else tmpB
            dst = flat[:, 0 : nseg * m].rearrange("p (n q) -> p n q", q=m)
            use_a = not use_a
        nc.vector.tensor_tensor(dst[:, :, 0:rem], cur[:, :, 0:rem], cur[:, :, m:cw], op=op)
        if m > rem:
            nc.vector.tensor_copy(dst[:, :, rem:m], cur[:, :, rem:m])
        cur, cw = dst, m


def _emit_bcast_mm(nc, z, c2h, ident, ccol, nseg, w, half, start):
    """Accumulate c2h[:, ccol+s] broadcast over window w into z[64h:64h+64, :],
    splitting matmuls at PSUM bank boundaries (512 cols). Last MM closes the
    accumulation group (stop=True)."""
    F = nseg * w
    p0 = 64 * half
    plan = []
    for c0 in range(0, F, 512):
        c1 = min(c0 + 512, F)
        s0, r0 = divmod(c0, w)
        pos = c0
        if r0:
            take = min(w - r0, c1 - c0)
            plan.append((pos, take, s0, 1, take))
            pos += take
            s0 += 1
        nfull = (c1 - pos) // w
        if nfull > 0:
            plan.append((pos, nfull * w, s0, nfull, w))
            pos += nfull * w
            s0 += nfull
        if pos < c1:
            take = c1 - pos
            plan.append((pos, take, s0, 1, take))
    for k, (pos, ncols, s0, nsegs, wk) in enumerate(plan):
        bc = c2h[p0 : p0 + 64, ccol + s0 : ccol + s0 + nsegs].to_broadcast([64, nsegs, wk])
        nc.tensor.matmul(z[p0 : p0 + 64, pos : pos + ncols], ident[p0 : p0 + 64, :], bc,
                         start=start, stop=(k == len(plan) - 1),
                         tile_position=(p0, p0), skip_group_check=True)


def _build_program(tiles, S, C):
    nc = bacc.Bacc("TRN2", target_bir_lowering=False, debug=False,
                   num_devices=NCORES)
    fp32, bf16 = mybir.dt.float32, mybir.dt.bfloat16
    Relu = mybir.ActivationFunctionType.Relu
    Copy = mybir.ActivationFunctionType.Copy

    mi_d = nc.dram_tensor("msgin", [32, S], bf16, kind="ExternalInput")
    wdefs = [("w1l1", [48, HID]), ("w1l2", [128, HID]), ("w1l3", [128, HID]),
             ("w2l1", [48, HID]), ("w2l2", [128, HID]), ("w2l3", [128, HID]),
             ("wh2", [128, HID]), ("identd", [128, HID])]
    wd = {n: nc.dram_tensor(n, s, bf16, kind="ExternalInput") for n, s in wdefs}
    bdefs = ["b11", "b21", "b31", "b12", "b22"]
    bd = {n: nc.dram_tensor(n, [128, 1], fp32, kind="ExternalInput") for n in bdefs}
    h1_o = nc.dram_tensor("h1", [128, C], bf16, kind="ExternalOutput")
    p2_o = nc.dram_tensor("p2", [128, C], fp32, kind="ExternalOutput")

    with tile.TileContext(nc) as tc:
        with tc.tile_pool(name="wp", bufs=1) as wp, \
             tc.tile_pool(name="per", bufs=1) as per, \
             tc.tile_pool(name="mip", bufs=MIP_BUFS) as mip, \
             tc.tile_pool(name="mp", bufs=MP_BUFS) as mp, \
             tc.tile_pool(name="trp", bufs=1) as trp, \
             tc.tile_pool(name="zp", bufs=ZP_BUFS, space="PSUM") as zp, \
             tc.tile_pool(name="cp", bufs=2, space="PSUM") as cp:
            wt = {}
            for n, s in wdefs:
                t = wp.tile(s, bf16, tag=n)
                nc.sync.dma_start(t[:, :], wd[n].ap())
                wt[n] = t
            bt = {}
            for n in bdefs:
                t = wp.tile([128, 1], fp32, tag=n)
                nc.sync.dma_start(t[:, :], bd[n].ap())
                bt[n] = t

            pst1 = per.tile([128, C], fp32, tag="pst1")
            h1t = per.tile([128, C], bf16, tag="h1t")
            c2h = per.tile([128, C], bf16, tag="c2h")
            pst2 = per.tile([128, C], fp32, tag="pst2")
            tmpA = trp.tile([128, FMAX], bf16, tag="tmpA")
            tmpB = trp.tile([128, FMAX], bf16, tag="tmpB")

            def conv(wl1, wl2, wl3, b1, b2, pst, use_c2h):
                for (w, nseg, ocol, ccol, _i0) in tiles:
                    F = nseg * w
                    mit = mip.tile([64, FMAX], bf16, tag="mi")
                    nc.sync.dma_start(mit[0:16, 0:F], mi_d.ap()[0:16, ocol : ocol + F])
                    nc.sync.dma_start(mit[32:48, 0:F], mi_d.ap()[16:32, ocol : ocol + F])
                    z1 = zp.tile([128, FMAX], fp32, tag="z")
                    for h in (0, 1):
                        for c0 in range(0, F, 512):
                            cw = min(512, F - c0)
                            nc.tensor.matmul(
                                z1[64 * h : 64 * h + 64, c0 : c0 + cw],
                                wl1[32 * h : 32 * h + 16, :],
                                mit[32 * h : 32 * h + 16, c0 : c0 + cw],
                                start=True, stop=not use_c2h,
                                tile_position=(32 * h, 64 * h), skip_group_check=True)
                    if use_c2h:
                        for h in (0, 1):
                            _emit_bcast_mm(nc, z1, c2h, wt["identd"], ccol, nseg, w, h, False)
                    m1 = mp.tile([128, FMAX], bf16, tag="m1")
                    nc.scalar.activation(m1[:, 0:F], z1[:, 0:F], Relu, bias=b1[:, 0:1])
                    z2 = zp.tile([128, FMAX], fp32, tag="z")
                    for h in (0, 1):
                        for c0 in range(0, F, 512):
                            cw = min(512, F - c0)
                            nc.tensor.matmul(
                                z2[64 * h : 64 * h + 64, c0 : c0 + cw],
                                wl2[64 * h : 64 * h + 64, :],
                                m1[64 * h : 64 * h + 64, c0 : c0 + cw],
                                start=True, stop=True,
                                tile_position=(64 * h, 64 * h), skip_group_check=True)
                    m2 = mp.tile([128, FMAX], bf16, tag="m2")
                    if RELU2_DVE_ALT and (ccol % 2 == 1):
                        nc.vector.scalar_tensor_tensor(
                            m2[:, 0:F], z2[:, 0:F], 1.0, b2[:, 0:1].to_broadcast([128, F]),
                            op0=mybir.AluOpType.mult, op1=mybir.AluOpType.add)
                        nc.vector.tensor_scalar_max(m2[:, 0:F], m2[:, 0:F], 0.0)
                    else:
                        nc.scalar.activation(m2[:, 0:F], z2[:, 0:F], Relu, bias=b2[:, 0:1])
                    z3 = zp.tile([128, FMAX], fp32, tag="z")
                    for h in (0, 1):
                        for c0 in range(0, F, 512):
                            cw = min(512, F - c0)
                            nc.tensor.matmul(
                                z3[64 * h : 64 * h + 64, c0 : c0 + cw],
                                wl3[64 * h : 64 * h + 64, :],
                                m2[64 * h : 64 * h + 64, c0 : c0 + cw],
                                start=True, stop=True,
                                tile_position=(64 * h, 64 * h), skip_group_check=True)
                    if SKIP_TREE:
                        nc.vector.tensor_copy(pst[:, ccol : ccol + nseg], z3[:, 0:nseg])
                    else:
                        z3s = mp.tile([128, FMAX], bf16, tag="z3s")
                        _emit_tree_max(nc, pst[:, ccol : ccol + nseg], z3[:, 0:F],
                                       z3s, tmpA, tmpB, nseg, w, use_act=(Z3_EVAC == "act" or (Z3_EVAC == "alt" and ccol % 2 == 0)))

            conv(wt["w1l1"][:, :], wt["w1l2"][:, :], wt["w1l3"][:, :],
                 bt["b11"], bt["b21"], pst1, False)
            # h1 = relu(pool + b3) (bf16), also DMA out
            nc.scalar.activation(h1t[:, :], pst1[:, :], Relu, bias=bt["b31"][:, 0:1])
            nc.sync.dma_start(h1_o.ap(), h1t[:, :])
            # c2h = Wh2.T @ h1 per node column
            for c0 in range(0, C, 512):
                cw = min(512, C - c0)
                zc = cp.tile([128, 512], fp32, tag="zc")
                for h in (0, 1):
                    nc.tensor.matmul(zc[64 * h : 64 * h + 64, 0:cw],
                                     wt["wh2"][64 * h : 64 * h + 64, :],
                                     h1t[64 * h : 64 * h + 64, c0 : c0 + cw],
                                     start=True, stop=True,
                                     tile_position=(64 * h, 64 * h), skip_group_check=True)
                nc.scalar.activation(c2h[:, c0 : c0 + cw], zc[:, 0:cw], Copy)
            if not SKIP_CONV2:
                conv(wt["w2l1"][:, :], wt["w2l2"][:, :], wt["w2l3"][:, :],
                     bt["b12"], bt["b22"], pst2, True)
            else:
                nc.vector.tensor_copy(pst2[:, 0:C], pst1[:, 0:C])
            nc.sync.dma_start(p2_o.ap(), pst2[:, :])
    nc.compile()
    return nc


def _prepare(x, pos, edge_index, batch):
    N = x.shape[0]
    src = np.asarray(edge_index[0], np.int64)
    dst = np.asarray(edge_index[1], np.int64)
    deg = np.bincount(dst, minlength=N)
    eorder = np.argsort(dst, kind="stable")
    estart = np.zeros(N + 1, np.int64)
    estart[1:] = np.cumsum(deg)
    nz = np.nonzero(deg > 0)[0]
    nzo = nz[np.argsort(-deg[nz], kind="stable")]
    core_nodes = [nzo[c::NCORES] for c in range(NCORES)]
    deg_by_core = [deg[cn] for cn in core_nodes]
    tiles, S, C = _schedule(deg_by_core)

    msgin = np.zeros((NCORES, 32, S), np.float32)
    # node_at: [core, half, C] -> node id or -1 (dup/virtual)
    node_at = np.full((NCORES, 2, C), -1, np.int64)
    for c in range(NCORES):
        cn = core_nodes[c]
        eids = np.full((2, S), -1, np.int64)
        for (w, nseg, ocol, ccol, i0) in tiles:
            avail = len(cn) - i0
            if avail <= 0:
                continue
            nA = min(nseg, avail)
            nB = min(nseg, max(0, avail - nseg))
            for half, nreal in ((0, nA), (1, nB)):
                if nreal == 0:
                    continue
                nodes = cn[i0 + half * nseg : i0 + half * nseg + nreal]
                node_at[c, half, ccol : ccol + nreal] = nodes
                padded = np.concatenate([nodes, np.repeat(nodes[-1:], nseg - nreal)])
                d = deg[padded]
                offs = np.minimum(np.arange(w)[None, :], (d - 1)[:, None])
                ids = eorder[estart[padded][:, None] + offs]  # [nseg, w]
                eids[half, ocol : ocol + nseg * w] = ids.ravel()
        mi = msgin[c]
        for half in (0, 1):
            valid = eids[half] >= 0
            e = eids[half, valid]
            dv, sv = dst[e], src[e]
            r = 16 * half
            mi[r + 0 : r + 3, valid] = pos[dv].T
            mi[r + 3 : r + 6, valid] = (pos[sv] - pos[dv]).T
            mi[r + 6 : r + 9, valid] = x[dv].T
    return tiles, S, C, msgin.astype(BF16), node_at, deg


def kernel(x, pos, edge_index, batch, params):
    x = np.asarray(x, FP32)
    pos = np.asarray(pos, FP32)
    batch = np.asarray(batch, np.int64)
    fold = _fold(params)
    tiles, S, C, msgin, node_at, deg = _prepare(x, pos, edge_index, batch)
    nc = _build_program(tiles, S, C)

    (W11, b11), (W12, b21), (W13, b31) = fold["conv1"]
    (W21, b12), (W22, b22), (W23, _b32_unused) = fold["conv2"]
    b32 = fold["conv2"][2][1]

    def pad16(W):
        out = np.zeros((48, HID), np.float32)
        out[: W.shape[0]] = W
        out[32 : 32 + W.shape[0]] = W
        return out.astype(BF16)

    def stack2(b):
        return np.concatenate([b, b]).astype(FP32).reshape(128, 1)

    def stackw(W):
        return np.concatenate([W, W], axis=0).astype(BF16)

    shared = {
        "w1l1": pad16(W11), "w1l2": stackw(W12), "w1l3": stackw(W13),
        "w2l1": pad16(W21[0:6]), "w2l2": stackw(W22), "w2l3": stackw(W23),
        "wh2": stackw(W21[6:70]),
        "identd": stackw(np.eye(HID, dtype=np.float32)),
        "b11": stack2(b11), "b21": stack2(b21), "b31": stack2(b31),
        "b12": stack2(b12), "b22": stack2(b22),
    }
    in_maps = [dict(shared, msgin=msgin[c]) for c in range(NCORES)]
    res = run_bass_kernel_spmd(nc, in_maps, core_ids=list(range(NCORES)))
    globals()["_LAST_RES"] = res
    globals()["_LAST_NC"] = nc
    globals()["_LAST_INMAPS"] = in_maps

    # host: mean pool + head
    B = int(batch.max()) + 1
    cnt = np.bincount(batch, minlength=B).astype(FP32)
    pool1 = np.zeros((B, HID), FP32)
    pool2 = np.zeros((B, HID), FP32)
    for c in range(NCORES):
        h1 = np.asarray(res.results[c]["h1"], dtype=FP32)
        p2 = np.asarray(res.results[c]["p2"], dtype=FP32)
        h2 = np.maximum(p2 + np.concatenate([b32, b32]).reshape(128, 1), 0.0)
        for half in (0, 1):
            sel = node_at[c, half] >= 0
            if not sel.any():
                continue
            g = batch[node_at[c, half, sel]]
            np.add.at(pool1, g, h1[64 * half : 64 * half + 64, sel].T)
            np.add.at(pool2, g, h2[64 * half : 64 * half + 64, sel].T)
    pool1 /= cnt[:, None]
    pool2 /= cnt[:, None]
    hcat = np.concatenate([pool1, pool2], axis=1)
    W1 = np.asarray(params["lin1"]["W"], FP32); bl1 = np.asarray(params["lin1"]["b"], FP32)
    W2 = np.asarray(params["lin2"]["W"], FP32); bl2 = np.asarray(params["lin2"]["b"], FP32)
    h = np.maximum(hcat @ W1 + bl1, 0.0)
    logits = h @ W2 + bl2
    logits = logits - logits.max(axis=1, keepdims=True)
    lse = np.log(np.exp(logits).sum(axis=1, keepdims=True))
    return (logits - lse).astype(FP32)


# revision 12
# speedup vs baseline: 2175.4769x; 1939.8571x over previous
# Trainium2 Bass kernel for nn_ASAP (gnn_message_passing).
# Strategy: edges are partitioned by destination node across 8 cores
# (round-robin by degree rank so all cores share one SPMD schedule).
# Message MLPs run on-device in feature-major [128, F] tiles (2 groups of
# 64 features); segment-max is a log-tree of tensor_tensor(max) ops over
# per-destination windows; the source contribution enters through a
# host-prepared per-edge feature array (pure function of edge_index), the
# destination-side hidden state h1 is broadcast per segment with an
# identity-matmul over a step-0 access pattern. BatchNorm (eval, gamma=1,
# beta=0) folds into the weights. Final mean-pool + 2-layer head run on
# host from the small per-core h tables.
import math
import numpy as np
import ml_dtypes

import concourse.bass as bass
import concourse.bacc as bacc
import concourse.mybir as mybir
import concourse.tile as tile
from concourse.bass_utils import run_bass_kernel_spmd
from concourse._compat import with_exitstack

NCORES = 8
HID = 64
FMAX = 1024
INV_STD = np.float32(1.0 / np.sqrt(1.0 + 1e-5))
BF16 = ml_dtypes.bfloat16
FP32 = np.float32
TRACE = False
_LAST_RES = None
# perf tunables (cost-model guided)
Z3_EVAC = "alt"      # "act" | "dve" | "alt"
RELU2_DVE_ALT = False  # alternate relu2 onto DVE for odd tiles
MP_BUFS = 3
ZP_BUFS = 3
MIP_BUFS = 4
REPEAT = 1
C2H_VIA_DVE = False
SKIP_CONV2 = False
SKIP_TREE = False
SKIP_L1MM = False


def _fold(params):
    out = {}
    for cv in ("conv1", "conv2"):
        blocks = []
        for blk in params[cv]:
            W = np.asarray(blk["W"], FP32) * INV_STD
            b = np.asarray(blk["b"], FP32) * INV_STD
            blocks.append((W, b))
        out[cv] = blocks
    return out


def _schedule(deg_by_core):
    """deg_by_core: list of per-core descending degree arrays.
    Returns (tiles, S, C): tiles = (w, nseg, ocol, ccol, i0)."""
    NN = max(len(d) for d in deg_by_core)
    prof = np.zeros(NN, np.int64)
    for d in deg_by_core:
        prof[: len(d)] = np.maximum(prof[: len(d)], d)
    tiles = []
    i, ocol, ccol = 0, 0, 0
    while i < NN:
        w = int(prof[i])
        rem = NN - i
        nseg = max(1, min(FMAX // w, (rem + 1) // 2))
        tiles.append((w, nseg, ocol, ccol, i))
        ocol += nseg * w
        ccol += nseg
        i += min(2 * nseg, rem)
    return tiles, ocol, ccol


def _emit_tree_max(nc, pool_out, zflat, z3s, tmpA, tmpB, nseg, w, use_act):
    """pool_out[128, nseg] = max over w of zflat [128, nseg*w] (PSUM).
    First evacuates PSUM to bf16 SBUF (TensorTensor cannot read 2 PSUM
    operands), then runs a log-tree of max ops on SBUF."""
    op = mybir.AluOpType.max
    F = nseg * w
    if w == 1:
        nc.vector.tensor_copy(pool_out, zflat[:, 0:F])
        return
    if use_act:
        nc.scalar.activation(z3s[:, 0:F], zflat[:, 0:F],
                             mybir.ActivationFunctionType.Copy)
    else:
        nc.vector.tensor_copy(z3s[:, 0:F], zflat[:, 0:F])
    src3 = z3s[:, 0:F].rearrange("p (n w) -> p n w", w=w)
    cur, cw, use_a = src3, w, True
    while cw > 1:
        m = 1 << (cw.bit_length() - 1)
        if m == cw:
            m //= 2
        rem = cw - m
        if m == 1:
            dst = pool_out.rearrange("p (n o) -> p n o", o=1)
        else:
            flat = tmpA if use_a else tmpB
            dst = flat[:, 0 : nseg * m].rearrange("p (n q) -> p n q", q=m)
            use_a = not use_a
        nc.vector.tensor_tensor(dst[:, :, 0:rem], cur[:, :, 0:rem], cur[:, :, m:cw], op=op)
        if m > rem:
            nc.vector.tensor_copy(dst[:, :, rem:m], cur[:, :, rem:m])
        cur, cw = dst, m


def _emit_bcast_mm(nc, z, c2h, ident, ccol, nseg, w, half, start):
    """Accumulate c2h[:, ccol+s] broadcast over window w into z[64h:64h+64, :],
    splitting matmuls at PSUM bank boundaries (512 cols). Last MM closes the
    accumulation group (stop=True)."""
    F = nseg * w
    p0 = 64 * half
    plan = []
    for c0 in range(0, F, 512):
        c1 = min(c0 + 512, F)
        s0, r0 = divmod(c0, w)
        pos = c0
        if r0:
            take = min(w - r0, c1 - c0)
            plan.append((pos, take, s0, 1, take))
            pos += take
            s0 += 1
        nfull = (c1 - pos) // w
        if nfull > 0:
            plan.append((pos, nfull * w, s0, nfull, w))
            pos += nfull * w
            s0 += nfull
        if pos < c1:
            take = c1 - pos
            plan.append((pos, take, s0, 1, take))
    for k, (pos, ncols, s0, nsegs, wk) in enumerate(plan):
        bc = c2h[p0 : p0 + 64, ccol + s0 : ccol + s0 + nsegs].to_broadcast([64, nsegs, wk])
        nc.tensor.matmul(z[p0 : p0 + 64, pos : pos + ncols], ident[p0 : p0 + 64, :], bc,
                         start=start, stop=(k == len(plan) - 1),
                         tile_position=(p0, p0), skip_group_check=True)


def _build_program(tiles, S, C):
    nc = bacc.Bacc("TRN2", target_bir_lowering=False, debug=False,
                   num_devices=NCORES)
    fp32, bf16 = mybir.dt.float32, mybir.dt.bfloat16
    Relu = mybir.ActivationFunctionType.Relu
    Copy = mybir.ActivationFunctionType.Copy

    mi_d = nc.dram_tensor("msgin", [32, S], bf16, kind="ExternalInput")
    wdefs = [("w1l1", [48, HID]), ("w1l2", [128, HID]), ("w1l3", [128, HID]),
             ("w2l1", [48, HID]), ("w2l2", [128, HID]), ("w2l3", [128, HID]),
             ("wh2", [128, HID]), ("identd", [128, HID])]
    wd = {n: nc.dram_tensor(n, s, bf16, kind="ExternalInput") for n, s in wdefs}
    bdefs = ["b11", "b21", "b31", "b12", "b22"]
    bd = {n: nc.dram_tensor(n, [128, 1], fp32, kind="ExternalInput") for n in bdefs}
    h1_o = nc.dram_tensor("h1", [128, C], bf16, kind="ExternalOutput")
    p2_o = nc.dram_tensor("p2", [128, C], fp32, kind="ExternalOutput")

    with tile.TileContext(nc) as tc:
        with tc.tile_pool(name="wp", bufs=1) as wp, \
             tc.tile_pool(name="per", bufs=1) as per, \
             tc.tile_pool(name="mip", bufs=MIP_BUFS) as mip, \
             tc.tile_pool(name="mp", bufs=MP_BUFS) as mp, \
             tc.tile_pool(name="trp", bufs=1) as trp, \
             tc.tile_pool(name="zp", bufs=ZP_BUFS, space="PSUM") as zp, \
             tc.tile_pool(name="cp", bufs=2, space="PSUM") as cp:
            wt = {}
            for n, s in wdefs:
                t = wp.tile(s, bf16, tag=n)
                nc.sync.dma_start(t[:, :], wd[n].ap())
                wt[n] = t
            bt = {}
            for n in bdefs:
                t = wp.tile([128, 1], fp32, tag=n)
                nc.sync.dma_start(t[:, :], bd[n].ap())
                bt[n] = t

            import contextlib
            rep_ctx = tc.For_i(0, REPEAT, 1) if REPEAT > 1 else contextlib.nullcontext()
            pst1 = per.tile([128, C], fp32, tag="pst1")
            h1t = per.tile([128, C], bf16, tag="h1t")
            c2h = per.tile([128, C], bf16, tag="c2h")
            pst2 = per.tile([128, C], fp32, tag="pst2")
            tmpA = trp.tile([128, FMAX], bf16, tag="tmpA")
            tmpB = trp.tile([128, FMAX], bf16, tag="tmpB")

            stk_rep = rep_ctx.__enter__()
            def conv(wl1, wl2, wl3, b1, b2, pst, use_c2h):
                for (w, nseg, ocol, ccol, _i0) in tiles:
                    F = nseg * w
                    mit = mip.tile([64, FMAX], bf16, tag="mi")
                    nc.sync.dma_start(mit[0:16, 0:F], mi_d.ap()[0:16, ocol : ocol + F])
                    nc.sync.dma_start(mit[32:48, 0:F], mi_d.ap()[16:32, ocol : ocol + F])
                    z1 = zp.tile([128, FMAX], fp32, tag="z")
                    for h in (0, 1):
                        for c0 in range(0, F, 512):
                            cw = min(512, F - c0)
                            nc.tensor.matmul(
                                z1[64 * h : 64 * h + 64, c0 : c0 + cw],
                                wl1[32 * h : 32 * h + 16, :],
                                mit[32 * h : 32 * h + 16, c0 : c0 + cw],
                                start=True, stop=(not use_c2h) or C2H_VIA_DVE,
                                tile_position=(32 * h, 64 * h), skip_group_check=True)
                    if use_c2h and not C2H_VIA_DVE:
                        for h in (0, 1):
                            _emit_bcast_mm(nc, z1, c2h, wt["identd"], ccol, nseg, w, h, False)
                    m1 = mp.tile([128, FMAX], bf16, tag="m1")
                    if use_c2h and C2H_VIA_DVE:
                        stt = trp.tile([128, FMAX], fp32, tag="sttt")
                        nc.vector.scalar_tensor_tensor(
                            stt[:, 0:F].rearrange("p (n w) -> p n w", w=w),
                            z1[:, 0:F].rearrange("p (n w) -> p n w", w=w),
                            b1[:, 0:1], c2h[:, ccol : ccol + nseg].to_broadcast([128, nseg, w]),
                            op0=mybir.AluOpType.add, op1=mybir.AluOpType.add)
                        nc.scalar.activation(m1[:, 0:F], stt[:, 0:F], Relu)
                    else:
                        nc.scalar.activation(m1[:, 0:F], z1[:, 0:F], Relu, bias=b1[:, 0:1])
                    z2 = zp.tile([128, FMAX], fp32, tag="z")
                    for h in (0, 1):
                        for c0 in range(0, F, 512):
                            cw = min(512, F - c0)
                            nc.tensor.matmul(
                                z2[64 * h : 64 * h + 64, c0 : c0 + cw],
                                wl2[64 * h : 64 * h + 64, :],
                                m1[64 * h : 64 * h + 64, c0 : c0 + cw],
                                start=True, stop=True,
                                tile_position=(64 * h, 64 * h), skip_group_check=True)
                    m2 = mp.tile([128, FMAX], bf16, tag="m2")
                    if RELU2_DVE_ALT and (ccol % 2 == 1):
                        nc.vector.scalar_tensor_tensor(
                            m2[:, 0:F], z2[:, 0:F], 1.0, b2[:, 0:1].to_broadcast([128, F]),
                            op0=mybir.AluOpType.mult, op1=mybir.AluOpType.add)
                        nc.vector.tensor_scalar_max(m2[:, 0:F], m2[:, 0:F], 0.0)
                    else:
                        nc.scalar.activation(m2[:, 0:F], z2[:, 0:F], Relu, bias=b2[:, 0:1])
                    z3 = zp.tile([128, FMAX], fp32, tag="z")
                    for h in (0, 1):
                        for c0 in range(0, F, 512):
                            cw = min(512, F - c0)
                            nc.tensor.matmul(
                                z3[64 * h : 64 * h + 64, c0 : c0 + cw],
                                wl3[64 * h : 64 * h + 64, :],
                                m2[64 * h : 64 * h + 64, c0 : c0 + cw],
                                start=True, stop=True,
                                tile_position=(64 * h, 64 * h), skip_group_check=True)
                    if SKIP_TREE:
                        nc.vector.tensor_copy(pst[:, ccol : ccol + nseg], z3[:, 0:nseg])
                    else:
                        z3s = mp.tile([128, FMAX], bf16, tag="z3s")
                        _emit_tree_max(nc, pst[:, ccol : ccol + nseg], z3[:, 0:F],
                                       z3s, tmpA, tmpB, nseg, w, use_act=(Z3_EVAC == "act" or (Z3_EVAC == "alt" and ccol % 2 == 0)))

            conv(wt["w1l1"][:, :], wt["w1l2"][:, :], wt["w1l3"][:, :],
                 bt["b11"], bt["b21"], pst1, False)
            # h1 = relu(pool + b3) (bf16), also DMA out
            nc.scalar.activation(h1t[:, :], pst1[:, :], Relu, bias=bt["b31"][:, 0:1])
            nc.sync.dma_start(h1_o.ap(), h1t[:, :])
            # c2h = Wh2.T @ h1 per node column
            for c0 in range(0, C, 512):
                cw = min(512, C - c0)
                zc = cp.tile([128, 512], fp32, tag="zc")
                for h in (0, 1):
                    nc.tensor.matmul(zc[64 * h : 64 * h + 64, 0:cw],
                                     wt["wh2"][64 * h : 64 * h + 64, :],
                                     h1t[64 * h : 64 * h + 64, c0 : c0 + cw],
                                     start=True, stop=True,
                                     tile_position=(64 * h, 64 * h), skip_group_check=True)
                nc.scalar.activation(c2h[:, c0 : c0 + cw], zc[:, 0:cw], Copy)
            if not SKIP_CONV2:
                conv(wt["w2l1"][:, :], wt["w2l2"][:, :], wt["w2l3"][:, :],
                     bt["b12"], bt["b22"], pst2, True)
            else:
                nc.vector.tensor_copy(pst2[:, 0:C], pst1[:, 0:C])
            nc.sync.dma_start(p2_o.ap(), pst2[:, :])
            rep_ctx.__exit__(None, None, None)
    nc.compile()
    return nc


def _prepare(x, pos, edge_index, batch):
    N = x.shape[0]
    src = np.asarray(edge_index[0], np.int64)
    dst = np.asarray(edge_index[1], np.int64)
    deg = np.bincount(dst, minlength=N)
    eorder = np.argsort(dst, kind="stable")
    estart = np.zeros(N + 1, np.int64)
    estart[1:] = np.cumsum(deg)
    nz = np.nonzero(deg > 0)[0]
    nzo = nz[np.argsort(-deg[nz], kind="stable")]
    core_nodes = [nzo[c::NCORES] for c in range(NCORES)]
    deg_by_core = [deg[cn] for cn in core_nodes]
    tiles, S, C = _schedule(deg_by_core)

    msgin = np.zeros((NCORES, 32, S), np.float32)
    # node_at: [core, half, C] -> node id or -1 (dup/virtual)
    node_at = np.full((NCORES, 2, C), -1, np.int64)
    for c in range(NCORES):
        cn = core_nodes[c]
        eids = np.full((2, S), -1, np.int64)
        for (w, nseg, ocol, ccol, i0) in tiles:
            avail = len(cn) - i0
            if avail <= 0:
                continue
            nA = min(nseg, avail)
            nB = min(nseg, max(0, avail - nseg))
            for half, nreal in ((0, nA), (1, nB)):
                if nreal == 0:
                    continue
                nodes = cn[i0 + half * nseg : i0 + half * nseg + nreal]
                node_at[c, half, ccol : ccol + nreal] = nodes
                padded = np.concatenate([nodes, np.repeat(nodes[-1:], nseg - nreal)])
                d = deg[padded]
                offs = np.minimum(np.arange(w)[None, :], (d - 1)[:, None])
                ids = eorder[estart[padded][:, None] + offs]  # [nseg, w]
                eids[half, ocol : ocol + nseg * w] = ids.ravel()
        mi = msgin[c]
        for half in (0, 1):
            valid = eids[half] >= 0
            e = eids[half, valid]
            dv, sv = dst[e], src[e]
            r = 16 * half
            mi[r + 0 : r + 3, valid] = pos[dv].T
            mi[r + 3 : r + 6, valid] = (pos[sv] - pos[dv]).T
            mi[r + 6 : r + 9, valid] = x[dv].T
    return tiles, S, C, msgin.astype(BF16), node_at, deg


def kernel(x, pos, edge_index, batch, params):
    x = np.asarray(x, FP32)
    pos = np.asarray(pos, FP32)
    batch = np.asarray(batch, np.int64)
    fold = _fold(params)
    tiles, S, C, msgin, node_at, deg = _prepare(x, pos, edge_index, batch)
    nc = _build_program(tiles, S, C)

    (W11, b11), (W12, b21), (W13, b31) = fold["conv1"]
    (W21, b12), (W22, b22), (W23, _b32_unused) = fold["conv2"]
    b32 = fold["conv2"][2][1]

    def pad16(W):
        out = np.zeros((48, HID), np.float32)
        out[: W.shape[0]] = W
        out[32 : 32 + W.shape[0]] = W
        return out.astype(BF16)

    def stack2(b):
        return np.concatenate([b, b]).astype(FP32).reshape(128, 1)

    def stackw(W):
        return np.concatenate([W, W], axis=0).astype(BF16)

    shared = {
        "w1l1": pad16(W11), "w1l2": stackw(W12), "w1l3": stackw(W13),
        "w2l1": pad16(W21[0:6]), "w2l2": stackw(W22), "w2l3": stackw(W23),
        "wh2": stackw(W21[6:70]),
        "identd": stackw(np.eye(HID, dtype=np.float32)),
        "b11": stack2(b11), "b21": stack2(b21), "b31": stack2(b31),
        "b12": stack2(b12), "b22": stack2(b22),
    }
    in_maps = [dict(shared, msgin=msgin[c]) for c in range(NCORES)]
    res = run_bass_kernel_spmd(nc, in_maps, core_ids=list(range(NCORES)))
    globals()["_LAST_RES"] = res
    globals()["_LAST_NC"] = nc
    globals()["_LAST_INMAPS"] = in_maps

    # host: mean pool + head
    B = int(batch.max()) + 1
    cnt = np.bincount(batch, minlength=B).astype(FP32)
    pool1 = np.zeros((B, HID), FP32)
    pool2 = np.zeros((B, HID), FP32)
    for c in range(NCORES):
        h1 = np.asarray(res.results[c]["h1"], dtype=FP32)
        p2 = np.asarray(res.results[c]["p2"], dtype=FP32)
        h2 = np.maximum(p2 + np.concatenate([b32, b32]).reshape(128, 1), 0.0)
        for half in (0, 1):
            sel = node_at[c, half] >= 0
            if not sel.any():
                continue
            g = batch[node_at[c, half, sel]]
            np.add.at(pool1, g, h1[64 * half : 64 * half + 64, sel].T)
            np.add.at(pool2, g, h2[64 * half : 64 * half + 64, sel].T)
    pool1 /= cnt[:, None]
    pool2 /= cnt[:, None]
    hcat = np.concatenate([pool1, pool2], axis=1)
    W1 = np.asarray(params["lin1"]["W"], FP32); bl1 = np.asarray(params["lin1"]["b"], FP32)
    W2 = np.asarray(params["lin2"]["W"], FP32); bl2 = np.asarray(params["lin2"]["b"], FP32)
    h = np.maximum(hcat @ W1 + bl1, 0.0)
    logits = h @ W2 + bl2
    logits = logits - logits.max(axis=1, keepdims=True)
    lse = np.log(np.exp(logits).sum(axis=1, keepdims=True))
    return (logits - lse).astype(FP32)
